# revision 58
# baseline (speedup 1.0000x reference)
"""Multi-head attention (B=4, S=2048, D=1024, H=16) on 8 trn2 cores.

Tensor-parallel over heads: each core owns 2 heads (128 channels).
Host: transpose q/k/v to [D, T]; q/k/Wq/Wk in fp8e4, v/Wv/Wo in bf16;
slice weights per core; sum bf16 partial outputs (+bo) on host.

Per-core dataflow (fp8 scores+QK-proj, transposed ctx, engine-balanced):
  - Q/K projections via fp8 DoubleRow matmuls (contract 256 D-rows per
    instr, 0.5 cyc/row); DVE adds bias and writes fp8e4 QpT8/KpT8
    [128c, 2, S]: slot 0 = data, slot 1 = zeros (DoubleRow zero-slot).
  - scores via fp8 DoubleRow matmul: lhsT=KpT8[64,2,128],
    rhs=QpT8[64,2,512] -> sc[128k, 512q] per head. The zeroed second slot
    contracts to 0, so cost halves without any data reshuffle.
  - exp: mostly ScalarE activation (scale=1/8 folded); k-blocks 5/10/15
    computed on DVE via Schraudolph bit-trick
    (int16_rne(x*23.083+16248.37) bitcast to bf16) to balance engines;
    those use ps_mm scores so the Act exp stream never WAR-waits on DVE.
  - ctx accumulated TRANSPOSED [q, d]: lhsT=attn[k,128q] (M=128 full),
    rhs=vex[k, 65] (V plus ones column -> denominator lands in psum col
    64 per head, per-PARTITION per q). 65 cyc/matmul vs 512 in [d,q]
    layout. PSUM zero-region: only the first accumulation group per
    2KB bank may set start=True.
  - normalize: DVE reciprocal of denoms + per-partition tensor_scalar
    mult -> ctxn [128q, dA|dB] bf16; DMA-transpose (xbar) -> ctxT[d, t].
  - out_partial = ctxT^T-slice @ Wo rows -> [t, 1024] f32 psum; DVE
    copies to bf16 staging; DMA partials to HBM (host sums + bo).

Schedule: exp-stream-paced software pipeline. Scores+exp run one k-block
ahead of the ctx accumulation (PE queue: sc(k+1) precedes ctx(k));
projections of batch b+1 and output projections of finished q-blocks
interleave as pullable filler units (1 unit of <=430ns PE per k-block:
in-order engine queues head-block on any not-ready instruction, so the
pull rate must stay at 1). Batch-boundary leftovers of A(b+1) drain
inside the first B-chunk's k-loop, per-chunk just before the first
score matmul that reads that chunk's K/V. Input DMAs prefetch a full
batch ahead on the Pool SWDGE queue; ctx transposes issue from SP.
"""

import numpy as np
import ml_dtypes

D = 1024
H = 16
B = 4
S = 2048
T = B * S  # 8192
NCORES = 8
CPC = D // NCORES  # 128 channels per core = 2 heads of 64
HD = 64  # head dim

_CACHE = {}

LAST_RESULTS = None  # BassKernelResults of the most recent run (for test.py)

# k-blocks whose exp runs on DVE (Schraudolph) instead of ScalarE.
SCHRAUD_KBS = (5, 10, 15)
# Schraudolph constants for exp(0.125*x) in bf16 bit space
SCH_MUL = 0.125 * 184.66496280094688
SCH_ADD = 16248.37


def _build_nc():
    import concourse.bass as bass
    import concourse.bacc as bacc
    import concourse.mybir as mybir
    import concourse.tile as tile
    from contextlib import ExitStack

    bf = mybir.dt.bfloat16
    f32 = mybir.dt.float32
    fp8 = mybir.dt.float8e4
    i16 = mybir.dt.int16
    DR = mybir.MatmulPerfMode.DoubleRow
    Exp = mybir.ActivationFunctionType.Exp
    MULT = mybir.AluOpType.mult
    ADD = mybir.AluOpType.add

    nc = bacc.Bacc("TRN2", target_bir_lowering=False, debug=False,
                   num_devices=NCORES)

    qT_d = nc.dram_tensor("qT", [D, T], fp8, kind="ExternalInput").ap()
    kT_d = nc.dram_tensor("kT", [D, T], fp8, kind="ExternalInput").ap()
    vT_d = nc.dram_tensor("vT", [D, T], bf, kind="ExternalInput").ap()
    wq_d = nc.dram_tensor("wq", [D, CPC], fp8, kind="ExternalInput").ap()
    wk_d = nc.dram_tensor("wk", [D, CPC], fp8, kind="ExternalInput").ap()
    wv_d = nc.dram_tensor("wv", [D, CPC], bf, kind="ExternalInput").ap()
    wo_d = nc.dram_tensor("wo", [CPC, D], bf, kind="ExternalInput").ap()
    bq_d = nc.dram_tensor("bq", [CPC, 1], f32, kind="ExternalInput").ap()
    bk_d = nc.dram_tensor("bk", [CPC, 1], f32, kind="ExternalInput").ap()
    bv_d = nc.dram_tensor("bv", [CPC, 1], f32, kind="ExternalInput").ap()
    out_d = nc.dram_tensor("out", [T, D], bf, kind="ExternalOutput").ap()

    NKT = D // 128          # 8 contraction tiles for projections
    NQB = S // 512          # 4 q-blocks per batch
    NKB = S // 128          # 16 k-blocks per batch
    NBT = S // 128          # 16 t-tiles per batch

    with ExitStack() as ctx:
        tc = ctx.enter_context(tile.TileContext(nc))

        const = ctx.enter_context(tc.tile_pool(name="const", bufs=1))
        res2 = ctx.enter_context(tc.tile_pool(name="res2", bufs=2))
        ctxTp = ctx.enter_context(tc.tile_pool(name="ctxTp", bufs=2))
        a_in = ctx.enter_context(tc.tile_pool(name="a_in", bufs=4))
        expp = ctx.enter_context(tc.tile_pool(name="expp", bufs=6))
        ctxnp = ctx.enter_context(tc.tile_pool(name="ctxnp", bufs=2))
        rcpp = ctx.enter_context(tc.tile_pool(name="rcpp", bufs=4))
        ostp = ctx.enter_context(tc.tile_pool(name="ostp", bufs=4))
        ps_mm = ctx.enter_context(tc.tile_pool(name="ps_mm", bufs=2, space="PSUM"))
        ps_sc = ctx.enter_context(tc.tile_pool(name="ps_sc", bufs=2, space="PSUM"))
        ps_ctxA = ctx.enter_context(tc.tile_pool(name="ps_ctxA", bufs=1, space="PSUM"))
        ps_ctxB = ctx.enter_context(tc.tile_pool(name="ps_ctxB", bufs=1, space="PSUM"))

        # ---- constants ----
        wq_sb = const.tile([128, NKT, CPC], fp8)
        nc.sync.dma_start(out=wq_sb, in_=wq_d.rearrange("(a p) c -> p a c", p=128))
        wk_sb = const.tile([128, NKT, CPC], fp8)
        nc.sync.dma_start(out=wk_sb, in_=wk_d.rearrange("(a p) c -> p a c", p=128))
        wv_sb = const.tile([128, NKT, CPC], bf)
        nc.sync.dma_start(out=wv_sb, in_=wv_d.rearrange("(a p) c -> p a c", p=128))
        wo_sb = const.tile([CPC, D], bf)
        nc.sync.dma_start(out=wo_sb, in_=wo_d)
        bq_sb = const.tile([CPC, 1], f32)
        nc.sync.dma_start(out=bq_sb, in_=bq_d)
        bk_sb = const.tile([CPC, 1], f32)
        nc.sync.dma_start(out=bk_sb, in_=bk_d)
        bv_bc = const.tile([128, CPC], f32)
        bv_bcast_ap = bass.AP(tensor=bv_d.tensor, offset=bv_d.offset,
                              ap=[[0, 128], [1, CPC]])
        nc.gpsimd.dma_start(out=bv_bc, in_=bv_bcast_ap)

        qT_r = qT_d.rearrange("(a p) t -> p a t", p=128)
        kT_r = kT_d.rearrange("(a p) t -> p a t", p=128)
        vT_r = vT_d.rearrange("(a p) t -> p a t", p=128)

        def alloc_batch_tiles():
            QpT8 = res2.tile([128, 2, S], fp8, tag="QpT8")
            KpT8 = res2.tile([128, 2, S], fp8, tag="KpT8")
            vex = res2.tile([128, NBT, 130], bf, tag="vex")
            # DoubleRow zero slots (must be 0, not garbage: NaN*0=NaN)
            nc.gpsimd.memset(QpT8[:, 1, :], 0.0)
            nc.gpsimd.memset(KpT8[:, 1, :], 0.0)
            nc.gpsimd.memset(vex[:, :, 64:65], 1.0)
            nc.gpsimd.memset(vex[:, :, 129:130], 1.0)
            return QpT8, KpT8, vex

        def prefetch_chunk(b, tb, skip_q=False):
            """Issue the input DMAs for one 512-token chunk of batch b.
            Batch 0 (pipeline warmup) loads on the idle SP/Act HWDGE queues
            -- the serial Pool SWDGE chain is too slow to stay ahead of the
            first batch's attention."""
            eng_q = nc.gpsimd
            eng_k = nc.gpsimd
            eng_v = nc.gpsimd
            tg = b * S + tb * 512
            qt = None
            if not skip_q:
                qt = a_in.tile([128, NKT, 512], fp8, tag="qt")
                eng_q.dma_start(out=qt, in_=qT_r[:, :, tg:tg + 512])
            ktile = a_in.tile([128, NKT, 512], fp8, tag="kt")
            eng_k.dma_start(out=ktile, in_=kT_r[:, :, tg:tg + 512])
            vt = a_in.tile([128, NKT, 512], bf, tag="vt")
            eng_v.dma_start(out=vt, in_=vT_r[:, :, tg:tg + 512])
            return qt, ktile, vt

        def a_chunk_units(b, tb, tiles, loaded):
            """Generator: projections for token block tb (512 tok) of batch b.
            Yields between matmul groups so B-phase bubbles can be filled."""
            QpT8, KpT8, vex = tiles
            qt, ktile, vt = loaded
            tl = tb * 512  # batch-local
            skip_q = qt is None
            yield
            if not skip_q:
                psq = ps_mm.tile([128, 512], f32, tag="mm")
                for j in range(NKT // 2):
                    nc.tensor.matmul(psq, lhsT=wq_sb[:, 2 * j:2 * j + 2, :],
                                     rhs=qt[:, 2 * j:2 * j + 2, :],
                                     start=(j == 0), stop=(j == NKT // 2 - 1),
                                     perf_mode=DR)
                    if j % 2 == 1:
                        yield
                nc.vector.tensor_scalar_add(QpT8[:, 0, tl:tl + 512], psq, bq_sb)
            yield
            psk = ps_mm.tile([128, 512], f32, tag="mm")
            for j in range(NKT // 2):
                nc.tensor.matmul(psk, lhsT=wk_sb[:, 2 * j:2 * j + 2, :],
                                 rhs=ktile[:, 2 * j:2 * j + 2, :],
                                 start=(j == 0), stop=(j == NKT // 2 - 1),
                                 perf_mode=DR)
                if j % 2 == 1:
                    yield
            nc.vector.tensor_scalar_add(KpT8[:, 0, tl:tl + 512], psk, bk_sb)
            yield
            for sub in range(4):
                tt = tb * 4 + sub  # batch-local t-tile
                psv = ps_mm.tile([128, 128], f32, tag="mm")
                for kt in range(NKT):
                    nc.tensor.matmul(
                        psv,
                        lhsT=vt[:, kt, sub * 128:(sub + 1) * 128],
                        rhs=wv_sb[:, kt, :],
                        start=(kt == 0), stop=(kt == NKT - 1))
                # one strided add covers both head halves of vex
                dst = vex[:, tt, :].rearrange("p (i c) -> p i c", c=65)[:, :, 0:64]
                src = psv.rearrange("p (i c) -> p i c", c=64)
                bcv = bv_bc.rearrange("p (i c) -> p i c", c=64)
                nc.vector.tensor_add(dst, src, bcv)
                yield

        def pull(fillers, n):
            done = 0
            while done < n and fillers:
                try:
                    next(fillers[0][1])
                    done += 1
                except StopIteration:
                    fillers.pop(0)

        def drain_a_upto(fillers, chunk, bb):
            """Exhaust pending A-unit generators of batch bb, chunks <= chunk."""
            for ent in list(fillers):
                kind, gen = ent[0], ent[1]
                if kind != "a" or ent[2] > chunk:
                    continue
                while True:
                    try:
                        next(gen)
                    except StopIteration:
                        break
                fillers.remove(ent)

        def drain_a_units(fillers, chunk, n, bb):
            """Pop up to n units from batch bb's A-generators, chunks <= chunk."""
            for ent in list(fillers):
                if n <= 0:
                    break
                kind, gen = ent[0], ent[1]
                if kind != "a" or ent[2] > chunk:
                    continue
                while n > 0:
                    try:
                        next(gen)
                        n -= 1
                    except StopIteration:
                        fillers.remove(ent)
                        break

        def emit_B_chunk(b, qb, tiles, ctxT, ctxn_t, fillers,
                         drain_batch=False, attn0=None):
            """Attention for one 512-wide q block of batch b (both heads).
            Pulls filler units (A/C-phase work) between k blocks. When
            drain_batch is set (first q-block of a new batch), leftover
            A-units of THIS batch are force-drained just before the first
            score matmul that reads their chunk's K/V data."""
            QpT8, KpT8, vex = tiles
            ql = qb * 512
            ctA = ps_ctxA.tile([128, 4 * 65], f32, tag="ctxA")
            ctB = ps_ctxB.tile([128, 4 * 65], f32, tag="ctxB")

            def score_exp(kb, ql=None):
                """Scores + exp for one k-block; returns the attn tile."""
                if ql is None:
                    ql = qb * 512
                kl = kb * 128
                schraud = kb in SCHRAUD_KBS or (kb == 2 and (b + qb) % 2)
                if schraud:
                    # DVE-exp k-blocks use ps_mm scores so the Act exp
                    # stream's psum recycling never WAR-waits on DVE.
                    e16 = expp.tile([128, 1024], i16, tag="exp")
                    for h in range(2):
                        sch = ps_mm.tile([128, 512], f32, tag="mm")
                        nc.tensor.matmul(sch,
                                         lhsT=KpT8[h * 64:(h + 1) * 64, :, kl:kl + 128],
                                         rhs=QpT8[h * 64:(h + 1) * 64, :, ql:ql + 512],
                                         start=True, stop=True, perf_mode=DR)
                        nc.vector.tensor_scalar(e16[:, h * 512:(h + 1) * 512],
                                                sch, SCH_MUL, SCH_ADD,
                                                op0=MULT, op1=ADD)
                    return e16.bitcast(bf)
                sc = ps_sc.tile([128, 1024], f32, tag="sc")
                nc.tensor.matmul(sc[:, 0:512],
                                 lhsT=KpT8[0:64, :, kl:kl + 128],
                                 rhs=QpT8[0:64, :, ql:ql + 512],
                                 start=True, stop=True, perf_mode=DR)
                nc.tensor.matmul(sc[:, 512:1024],
                                 lhsT=KpT8[64:128, :, kl:kl + 128],
                                 rhs=QpT8[64:128, :, ql:ql + 512],
                                 start=True, stop=True, perf_mode=DR)
                eAB = expp.tile([128, 1024], bf, tag="exp")
                nc.scalar.activation(eAB, sc, Exp, scale=0.125)
                return eAB

            # software-pipelined: scores/exp run one k-block ahead so the
            # PE has independent work queued while exp(kb) is in flight
            if drain_batch:
                drain_a_upto(fillers, 0, b)
            attn = attn0 if attn0 is not None else score_exp(0)
            pull(fillers, 2)
            pre_attn = None
            for kb in range(NKB):
                if drain_batch and kb + 1 < NKB:
                    drain_a_units(fillers, min((kb + 4) // 4, 3), 1, b)
                    drain_a_upto(fillers, (kb + 1) // 4, b)
                if kb + 1 < NKB:
                    attn_next = score_exp(kb + 1)
                elif qb + 1 < NQB:
                    # pre-emit the NEXT q-block's first scores+exp so the
                    # Act stream never waits for this chunk's ctx drain
                    pre_attn = score_exp(0, ql=(qb + 1) * 512)
                    attn_next = None
                else:
                    attn_next = None
                pull(fillers, 1)
                for h, ct in ((0, ctA), (1, ctB)):
                    for qt in range(4):
                        # start=True marks the whole 2KB psum zero-region
                        # pending-zero, so only the FIRST group of each tile
                        # may set it; later groups' first writes land on
                        # pending-zero bytes and replace (= accumulate on 0).
                        nc.tensor.matmul(
                            ct[:, qt * 65:(qt + 1) * 65],
                            lhsT=attn[:, h * 512 + qt * 128:h * 512 + (qt + 1) * 128],
                            rhs=vex[:, kb, h * 65:(h + 1) * 65],
                            start=(kb == 0 and qt == 0),
                            stop=(kb == NKB - 1),
                            skip_group_check=True)
                attn = attn_next
            # normalize: denom is psum col 64 of each 65-wide group; one
            # broadcast-mult per head (rcp [128,4] stride-0-broadcast over d)
            for h, ct in ((0, ctA), (1, ctB)):
                rcp = rcpp.tile([128, 4], f32, tag="rcp")
                ctv = ct.rearrange("p (i c) -> p i c", c=65)
                nc.vector.reciprocal(rcp, ctv[:, :, 64:65])
                rcp_bc = bass.AP(tensor=rcp.tensor, offset=rcp.offset,
                                 ap=[rcp.ap[0], [1, 4], [0, 64]])
                nc.vector.tensor_tensor(ctxn_t[:, :, h * 64:(h + 1) * 64],
                                        ctv[:, :, 0:64], rcp_bc,
                                        op=MULT)
            for qt in range(4):
                nc.sync.dma_start_transpose(
                    out=ctxT[:, ql + qt * 128:ql + (qt + 1) * 128],
                    in_=ctxn_t[:, qt, :])
            return pre_attn

        def c_chunk_units(b, qb, ctxT):
            """Generator: output projection for q block qb of batch b."""
            for pair in range(2):
                ost = ostp.tile([128, 2, D], bf, tag="ost")
                for sub in range(2):
                    tt = qb * 4 + pair * 2 + sub
                    for eh in range(2):
                        po = ps_mm.tile([128, 512], f32, tag="mm")
                        nc.tensor.matmul(
                            po, lhsT=ctxT[:, tt * 128:(tt + 1) * 128],
                            rhs=wo_sb[:, eh * 512:(eh + 1) * 512],
                            start=True, stop=True)
                        yield
                        nc.vector.tensor_copy(
                            ost[:, sub, eh * 512:(eh + 1) * 512], po)
                        yield
                tg = b * S + (qb * 4 + pair * 2) * 128
                nc.sync.dma_start(
                    out=out_d[tg:tg + 256, :].rearrange("(a p) d -> p a d", p=128),
                    in_=ost)
                yield

        # software pipeline: A-projections of batch b+1 and C-output of the
        # previous q block interleave INTO the kb loop of B(b, qb)
        tiles = alloc_batch_tiles()
        # first q-tile load split per contraction tile so the very first
        # matmul only waits for a 128KB DMA, not a 1MB one
        qt0 = a_in.tile([128, NKT, 512], fp8, tag="qt")
        for kt in range(NKT):
            nc.sync.dma_start(out=qt0[:, kt, :], in_=qT_r[:, kt, 0:512])
        psq0 = ps_mm.tile([128, 512], f32, tag="mm")
        for j in range(NKT // 2):
            nc.tensor.matmul(psq0, lhsT=wq_sb[:, 2 * j:2 * j + 2, :],
                             rhs=qt0[:, 2 * j:2 * j + 2, :],
                             start=(j == 0), stop=(j == NKT // 2 - 1),
                             perf_mode=DR)
        nc.vector.tensor_scalar_add(tiles[0][:, 0, 0:512], psq0, bq_sb)
        ld = [prefetch_chunk(0, 0, skip_q=True)]
        ld += [prefetch_chunk(0, tb) for tb in range(1, NQB)]
        for tb in range(NQB):
            for _ in a_chunk_units(0, tb, tiles, ld[tb]):
                pass
        fillers = []
        for b in range(B):
            ctxT = ctxTp.tile([128, S], bf, tag="ctxT")
            next_tiles = alloc_batch_tiles() if b + 1 < B else None
            if next_tiles is not None:
                next_ld = [prefetch_chunk(b + 1, tb) for tb in range(NQB)]
            attn0 = None
            for qb in range(NQB):
                ctxn_t = ctxnp.tile([128, 4, 128], bf, tag="ctxn",
                                    name="ctxn")
                if next_tiles is not None:
                    fillers.append(("a", a_chunk_units(b + 1, qb, next_tiles,
                                                       next_ld[qb]), qb, b + 1))
                attn0 = emit_B_chunk(b, qb, tiles, ctxT, ctxn_t, fillers,
                                     drain_batch=(qb == 0 and b > 0),
                                     attn0=attn0)
                fillers.append(("c", c_chunk_units(b, qb, ctxT), None, None))
            tiles = next_tiles
        while fillers:
            pull(fillers, 100)

    nc.compile()
    return nc


def _get_nc():
    if "nc" not in _CACHE:
        _CACHE["nc"] = _build_nc()
    return _CACHE["nc"]


def kernel(q, k, v, mask, Wq, bq, Wk, bk, Wv, bv, Wo, bo):
    global LAST_RESULTS
    import os
    from concourse.bass_utils import run_bass_kernel_spmd

    bf16 = ml_dtypes.bfloat16
    f32 = np.float32

    q = np.asarray(q, dtype=f32).reshape(T, D)
    k = np.asarray(k, dtype=f32).reshape(T, D)
    v = np.asarray(v, dtype=f32).reshape(T, D)
    Wq = np.asarray(Wq, dtype=f32)
    Wk = np.asarray(Wk, dtype=f32)
    Wv = np.asarray(Wv, dtype=f32)
    Wo = np.asarray(Wo, dtype=f32)
    bq = np.asarray(bq, dtype=f32)
    bk = np.asarray(bk, dtype=f32)
    bv = np.asarray(bv, dtype=f32)
    bo = np.asarray(bo, dtype=f32)

    fp8 = ml_dtypes.float8_e4m3
    qT = np.ascontiguousarray(q.T).astype(bf16).astype(fp8)
    kT = np.ascontiguousarray(k.T).astype(bf16).astype(fp8)
    vT = np.ascontiguousarray(v.T).astype(bf16)

    in_maps = []
    for c in range(NCORES):
        sl = slice(c * CPC, (c + 1) * CPC)
        in_maps.append({
            "qT": qT, "kT": kT, "vT": vT,
            "wq": np.ascontiguousarray(Wq[:, sl]).astype(bf16).astype(fp8),
            "wk": np.ascontiguousarray(Wk[:, sl]).astype(bf16).astype(fp8),
            "wv": np.ascontiguousarray(Wv[:, sl]).astype(bf16),
            "wo": np.ascontiguousarray(Wo[sl, :]).astype(bf16),
            "bq": np.ascontiguousarray(bq[sl]).reshape(CPC, 1),
            "bk": np.ascontiguousarray(bk[sl]).reshape(CPC, 1),
            "bv": np.ascontiguousarray(bv[sl]).reshape(CPC, 1),
        })

    nc = _get_nc()
    trace = bool(int(os.environ.get("MHA_TRACE", "0")))
    LAST_RESULTS = run_bass_kernel_spmd(nc, in_maps, list(range(NCORES)),
                                        trace=trace)
    acc = np.zeros((T, D), f32)
    for r in LAST_RESULTS.results:
        acc += np.asarray(r["out"], dtype=f32)
    acc += bo
    return acc.reshape(B, S, D)
